# revision 5
# baseline (speedup 1.0000x reference)
"""Canny edge detection (nn_Canny) — hand-written Bass/Tile kernel for 8 trn2 cores.

Data-parallel: batch dim (8 images) sharded 1 image/core. Each core runs the
full Canny pipeline on its 1024x1024 fp32 image entirely in SBUF:

  gauss3x3 -> sobel -> grad mag/angle-bucket masks -> directional NMS
  (float-equality, matching the reference's leaky per-channel OR semantics)
  -> 50/80 double threshold -> 3x iterative 5x5 hysteresis -> binary out.

Layout ("strip"): one SBUF tile [128 partitions x 8336 fp32] per full-image
tensor. Partition p holds rows 8p..8p+7; row r of a partition lives at free
offset 16 + 1040*r, 1024 valid cols, with 16-col zero gaps between rows so
horizontal +-shifts read zeros at image edges. Vertical shifts are free-dim
offsets of +-1040 for 7/8 rows plus small cross-partition boundary
instructions for the row that crosses a partition.

Everything through the NMS equality tests runs in fp32 (the reference's
float-equality NMS and 50/80 thresholds are too tie-sensitive for 16-bit).
The hysteresis iterations run on an exact {0, 0.5, 1} encoding in bf16.

All constants are folded so the computed field is the reference's field
scaled by 1/a^2 (a = gaussian 1-D corner weight); thresholds 50/80/255 are
scaled to match. The final output is binary so the scale never materializes.
"""

import numpy as np

N_CORES = 8
H = W = 1024
P = 128        # SBUF partitions
S = 8          # row-slots per partition (rows 8p..8p+7)
RS = 1040      # row stride in the strip free dim
ORIG = 16      # first data col (gap of 16 zero cols before each row)
F = ORIG + S * RS  # 8336 free elems/partition; row r data at [16+1040r, 16+1040r+1024)

_SIGMA = 0.8
_gvec = np.exp(-(np.arange(-1.0, 2.0) ** 2) / (2.0 * _SIGMA**2))
_g1 = _gvec / _gvec.sum()          # [a, b, a]
GA, GB = float(_g1[0]), float(_g1[1])
C_BA = float(np.float32(GB / GA))  # b/a
_ALPHA = GA * GA                   # computed field = reference field / alpha
_T1 = np.tan(np.deg2rad(22.5))
_T2 = np.tan(np.deg2rad(67.5))
K1 = float(np.float32(1.0 + _T1 * _T1))
K2 = float(np.float32(1.0 + _T2 * _T2))
C255 = float(np.float32(255.0 / _ALPHA))
C50 = float(np.float32(50.0 / _ALPHA))
C80 = float(np.float32(80.0 / _ALPHA))

_cache = {}


def _build():
    from contextlib import ExitStack
    from concourse import bacc, tile
    import concourse.mybir as mybir

    dt32 = mybir.dt.float32
    dt16 = mybir.dt.bfloat16
    A = mybir.AluOpType

    nc = bacc.Bacc("TRN2", target_bir_lowering=False, debug=False,
                   num_devices=N_CORES)
    x_d = nc.dram_tensor("x", [H, W], dt32, kind="ExternalInput").ap()
    o_d = nc.dram_tensor("out", [H, W], dt32, kind="ExternalOutput").ap()

    with tile.TileContext(nc) as tc, ExitStack() as ctx:
        pool = ctx.enter_context(tc.tile_pool(name="p", bufs=1))

        def strip(tag, dtype=dt32, bufs=1):
            return pool.tile([P, F], dtype, tag=tag, bufs=bufs)

        def zero_gaps(t):
            # gaps at [1040k, 1040k+16), k=0..8
            nc.vector.memset(
                t[:, 0:S * RS].rearrange("p (k c) -> p k c", k=S, c=RS)[:, :, 0:ORIG],
                0.0,
            )
            nc.vector.memset(t[:, S * RS:F], 0.0)

        def span(t, p0, p1, rs, re, d=0):
            """AP over partitions [p0,p1), slots rs..re incl, flat shift d."""
            return t[p0:p1, ORIG + rs * RS + d: ORIG + re * RS + W + d]

        def tt(out, a, b, op, sa=(0, 0), sb=(0, 0)):
            """out = op(a shifted sa, b shifted sb); sa/sb = (dy,dx), |dy|<=1."""
            da, db = sa[0], sb[0]
            runs = []
            cur = None
            for r in range(S):
                q = ((r + da) // S, (r + db) // S)
                if cur is not None and cur[2] == q:
                    cur[1] = r
                else:
                    cur = [r, r, q]
                    runs.append(cur)
            for rs_, re_, (qa, qb) in runs:
                plo = 1 if (qa < 0 or qb < 0) else 0
                phi = (P - 1) if (qa > 0 or qb > 0) else P
                if phi > plo:
                    nc.vector.tensor_tensor(
                        span(out, plo, phi, rs_, re_),
                        span(a, plo + qa, phi + qa, rs_ + da - S * qa,
                             re_ + da - S * qa, sa[1]),
                        span(b, plo + qb, phi + qb, rs_ + db - S * qb,
                             re_ + db - S * qb, sb[1]),
                        op=op,
                    )
                # image-edge rows: the missing-source partition
                for pe, (qm, qp) in ((0, (qa, qb)), (P - 1, (qa, qb))):
                    if pe == 0 and not (qa < 0 or qb < 0):
                        continue
                    if pe == P - 1 and not (qa > 0 or qb > 0):
                        continue
                    # exactly one source is present (q == 0 side)
                    if pe == 0:
                        pres, dpres, spres = ((b, db, sb[1]) if qa < 0
                                              else (a, da, sa[1]))
                        amissing = qa < 0
                    else:
                        pres, dpres, spres = ((b, db, sb[1]) if qb > 0
                                              else (a, da, sa[1]))
                        amissing = qb > 0
                    o_ap = span(out, pe, pe + 1, rs_, re_)
                    p_ap = span(pres, pe, pe + 1, rs_ + dpres, re_ + dpres, spres)
                    if op == A.add:
                        nc.vector.tensor_copy(o_ap, p_ap)
                    elif op == A.subtract:
                        if amissing:  # out = 0 - b
                            nc.vector.tensor_scalar_mul(o_ap, p_ap, -1.0)
                        else:         # out = a - 0
                            nc.vector.tensor_copy(o_ap, p_ap)
                    elif op == A.max:
                        nc.vector.tensor_scalar_max(o_ap, p_ap, 0.0)
                    else:
                        raise NotImplementedError(str(op))

        def ts(out, in_, s1, op0, s2=None, op1=None, rng=None):
            o = out[:, 0:F] if rng is None else out[:, rng[0]:rng[1]]
            i = in_[:, 0:F] if rng is None else in_[:, rng[0]:rng[1]]
            if s2 is None:
                nc.vector.tensor_scalar(o, i, s1, None, op0)
            else:
                nc.vector.tensor_scalar(o, i, s1, s2, op0, op1)

        # ---------- load input ----------
        xs = strip("x")
        zero_gaps(xs)
        nc.sync.dma_start(
            xs[:, ORIG:F].rearrange("p (a c) -> p a c", a=S, c=RS)[:, :, 0:W],
            x_d.rearrange("(p a) w -> p a w", p=P, a=S),
        )

        # ---------- gaussian (scaled by 1/a per 1-D pass) ----------
        u = strip("u")
        tt(u, xs, xs, A.add, sa=(0, -1), sb=(0, 1))
        tx = strip("tx")
        ts(tx, xs, C_BA, A.mult)
        h = strip("h")
        tt(h, u, tx, A.add)
        v = strip("v")
        tt(v, h, h, A.add, sa=(-1, 0), sb=(1, 0))
        th = strip("th")
        ts(th, h, C_BA, A.mult)
        s_ = strip("s")
        tt(s_, v, th, A.add)
        zero_gaps(s_)

        # ---------- sobel ----------
        t1 = strip("t1")
        tt(t1, s_, s_, A.add, sa=(-1, 0), sb=(1, 0))
        ts2 = strip("ts2")
        ts(ts2, s_, 2.0, A.mult)
        t2 = strip("t2")
        tt(t2, t1, ts2, A.add)
        zero_gaps(t2)
        gx = strip("gx")
        tt(gx, t2, t2, A.subtract, sa=(0, 1), sb=(0, -1))
        q1 = strip("q1")
        tt(q1, s_, s_, A.add, sa=(0, -1), sb=(0, 1))
        q2 = strip("q2")
        tt(q2, q1, ts2, A.add)
        gy = strip("gy")
        tt(gy, q2, q2, A.subtract, sa=(-1, 0), sb=(1, 0))

        # ---------- magnitude + angle masks ----------
        xx = strip("xx")
        nc.scalar.square(xx[:, 0:F], gx[:, 0:F])
        yy = strip("yy")
        nc.scalar.square(yy[:, 0:F], gy[:, 0:F])
        m2 = strip("m2")
        tt(m2, xx, yy, A.add)
        pp = strip("pp")
        tt(pp, gx, gy, A.mult)
        mag = strip("mag")
        nc.scalar.sqrt(mag[:, 0:F], m2[:, 0:F])
        zk = strip("zk")
        ts(zk, yy, K1, A.mult)
        z = strip("z")
        tt(z, m2, zk, A.subtract)
        wk = strip("wk")
        ts(wk, yy, K2, A.mult)
        w = strip("w")
        tt(w, m2, wk, A.subtract)
        magc = strip("magc")
        ts(magc, mag, C255, A.min)
        m0 = strip("m0")
        ts(m0, z, 0.0, A.is_le)
        r0 = strip("r0")
        tt(r0, m0, magc, A.mult)
        zero_gaps(r0)
        m2m = strip("m2m")
        ts(m2m, w, 0.0, A.is_ge)
        r2 = strip("r2")
        tt(r2, m2m, magc, A.mult)
        q0 = strip("q0")
        tt(q0, magc, r0, A.subtract)
        rm = strip("rm")
        tt(rm, q0, r2, A.subtract)
        neg = strip("neg")
        ts(neg, pp, 0.0, A.is_lt)
        r1 = strip("r1")
        tt(r1, neg, rm, A.mult)
        zero_gaps(r1)
        r3 = strip("r3")
        tt(r3, rm, r1, A.subtract)
        zero_gaps(r3)

        # ---------- NMS (leaky per-channel OR, float equality) ----------
        offs = {0: (0, 1), 1: (-1, 1), 2: (-1, 0), 3: (-1, -1)}
        Vs = []
        for c, r in enumerate([r0, r1, r2, r3]):
            dy, dx = offs[c]
            d = strip(f"d{c}")
            tt(d, r, r, A.max, sa=(dy, dx), sb=(-dy, -dx))
            Vc = strip(f"V{c}")
            tt(Vc, d, r, A.is_le)
            Vs.append(Vc)
        o1 = strip("o1")
        tt(o1, Vs[0], Vs[1], A.max)
        o2 = strip("o2")
        tt(o2, Vs[2], Vs[3], A.max)
        anyeq = strip("anyeq")
        tt(anyeq, o1, o2, A.max)
        edge = strip("edge")
        tt(edge, anyeq, magc, A.mult)

        # ---------- double threshold -> e in {0, 0.5, 1} (bf16) ----------
        e1 = strip("e1", dt16)
        ts(e1, edge, C50, A.is_ge, 0.5, A.mult)
        e2 = strip("e2", dt16)
        ts(e2, edge, C80, A.is_ge, 0.5, A.mult)
        e = strip("e", dt16, bufs=2)
        tt(e, e1, e2, A.add)
        zero_gaps(e)

        # ---------- hysteresis: 3 iters of 5x5 dilate + weak bump ----------
        for it in range(3):
            mv = strip("mv", dt16, bufs=2)
            tt(mv, e, e, A.max, sa=(-1, 0), sb=(1, 0))
            m3v = strip("m3v", dt16, bufs=2)
            tt(m3v, mv, e, A.max)
            v5 = strip("v5", dt16, bufs=2)
            tt(v5, m3v, m3v, A.max, sa=(-1, 0), sb=(1, 0))
            zero_gaps(v5)
            mh = strip("mh", dt16, bufs=2)
            tt(mh, v5, v5, A.max, sa=(0, -1), sb=(0, 1))
            m3h = strip("m3h", dt16, bufs=2)
            tt(m3h, mh, v5, A.max)
            zero_gaps(m3h)
            Pd = strip("Pd", dt16, bufs=2)
            tt(Pd, m3h, m3h, A.max, sa=(0, -1), sb=(0, 1))
            up = strip("up", dt16, bufs=2)
            ts(up, Pd, 1.0, A.is_ge)
            bump = strip("bump", dt16, bufs=2)
            tt(bump, e, up, A.mult)
            en = strip("e", dt16, bufs=2)
            tt(en, e, bump, A.add)
            ts(en, en, 1.0, A.min)
            zero_gaps(en)
            e = en

        # ---------- binarize + store ----------
        ob = strip("ob")
        ts(ob, e, 1.0, A.is_ge)
        nc.sync.dma_start(
            o_d.rearrange("(p a) w -> p a w", p=P, a=S),
            ob[:, ORIG:F].rearrange("p (a c) -> p a c", a=S, c=RS)[:, :, 0:W],
        )

    nc.compile()
    return nc


def _get_nc():
    if "nc" not in _cache:
        _cache["nc"] = _build()
    return _cache["nc"]


def kernel(x, gaussian_kernel, sobel_kernel):
    from concourse.bass_utils import run_bass_kernel_spmd

    x = np.ascontiguousarray(np.asarray(x, dtype=np.float32).reshape(N_CORES, H, W))
    nc = _get_nc()
    in_maps = [{"x": x[i]} for i in range(N_CORES)]
    res = run_bass_kernel_spmd(nc, in_maps, core_ids=list(range(N_CORES)))
    out = np.stack([res.results[i]["out"] for i in range(N_CORES)])
    return out.reshape(N_CORES, H, W, 1).astype(np.float32)
